# revision 50
# baseline (speedup 1.0000x reference)
"""Multi-head causal attention (QKV proj + RoPE + softmax) on 8 TRN2 NeuronCores.

Sharding: batch 4-way x head-group 2-way -> each core handles 1 batch and 8
contiguous heads (512 output channels). No collectives; host gathers slices.

Per-core algorithm (all matmul compute in bf16, fp32 PSUM accumulation):
  - host passes x.T (q/k/v of its batch, transposed to [emb, seq]) and W.T
    shards so every matmul contracts over the partition dim without on-device
    transposes.
  - q/k weights are row-permuted per head into [even dims | odd dims] so RoPE
    becomes: rot = x*cs + swap32(x)*sn, where swap32 is an SBUF partition-block
    swap done by DMA. The per-head dim permutation cancels in q.k dot products.
  - q/k biases are per-partition columns folded into the PSUM eviction
    (tensor_scalar add); the v bias is applied on host: P@(V+b) = P@V + l*b.
  - scores are computed transposed, S_T[k, q] = kh_T.T @ qh_T (K=64
    contraction; the A/B heads of a 128-row tile are emitted adjacently so
    they run concurrently on PE row groups).
  - softmax: exp on ScalarE from PSUM at [128,1024] granularity (no max
    subtraction: |scores| <= ~5 by construction), causal mask multiplies on
    DVE for the diagonal tiles only; fully-masked k-tiles are skipped.
  - attnT[d, q] = sum_kt V_tile[k,d|1].T @ P_T[k, q] -- a ones-column appended
    to V makes row 64 the softmax denominator for free.
  - unnormalized attnT and the denominator row go to HBM via one SBUF staging
    copy; division + final transpose + v-bias happen on host.

Scheduling (the perf rewrite vs the 313us baseline): instead of projections
-> attention phases, everything is one software pipeline. Attention units
start as soon as the m=0 projections land (~19us), and the remaining
projection pieces (8 matmul-pair quanta) are woven between the per-kt
score/exp/attnV steps so the PE never idles while ScalarE paces the exp
stream. DMA issue is split across the sync/gpsimd/vector queues so no
engine serializes input loading.
"""

import sys
import types
from collections import deque

import numpy as np
import ml_dtypes

BF16 = ml_dtypes.bfloat16
SEQ, EMB, NHEADS, BATCH = 2048, 1024, 16, 4
HD, HALF = 64, 32
HPC = 8          # heads per core
DH = 512         # output dims per core
NE = EMB // 128  # 8 contraction tiles
NT = 4           # head-pair (128-row) dout tiles
NKT = SEQ // 128  # 16 key tiles
NQC = SEQ // 512  # 4 query chunks


def _install_ntff_shim():
    """The image's antenv lacks axon_hooks; synthesize it from trn_agent_boot
    so run_bass_kernel_spmd(trace=True) can profile. Harmless if unused."""
    try:
        import antenv.axon_hooks  # noqa: F401
        return
    except ImportError:
        pass
    try:
        from trn_agent_boot.trn_boot import _ntff_profile_via_ctypes
        import antenv
    except ImportError:
        return
    hook = _ntff_profile_via_ctypes("/opt/axon/libaxon_pjrt.so")
    mod = types.ModuleType("antenv.axon_hooks")
    mod.get_axon_ntff_profile_hook = lambda: hook
    mod.set_axon_ntff_profile_hook = lambda h: None
    sys.modules["antenv.axon_hooks"] = mod
    antenv.axon_hooks = mod


_built = {}


def build(causal=True):
    if causal in _built:
        return _built[causal]
    import concourse.mybir as mybir
    import concourse.tile as tile
    from concourse import bacc

    f32 = mybir.dt.float32
    bf = mybir.dt.bfloat16
    EXP = mybir.ActivationFunctionType.Exp
    IDN = mybir.ActivationFunctionType.Identity
    CPY = mybir.ActivationFunctionType.Copy
    MUL = mybir.AluOpType.mult
    ADD = mybir.AluOpType.add

    nc = bacc.Bacc(None, target_bir_lowering=False, debug=False)
    with tile.TileContext(nc) as tc:
        with tc.tile_pool(name="dram", bufs=1, space="DRAM") as dram:
            xq_d = dram.tile([EMB, SEQ], bf, kind="ExternalInput", name="xq", uniquify=False)
            xk_d = dram.tile([EMB, SEQ], bf, kind="ExternalInput", name="xk", uniquify=False)
            xv_d = dram.tile([EMB, SEQ], bf, kind="ExternalInput", name="xv", uniquify=False)
            wq_d = dram.tile([EMB, DH], bf, kind="ExternalInput", name="wq", uniquify=False)
            wk_d = dram.tile([EMB, DH], bf, kind="ExternalInput", name="wk", uniquify=False)
            wv_d = dram.tile([EMB, DH], bf, kind="ExternalInput", name="wv", uniquify=False)
            bqc_d = dram.tile([128, NT], f32, kind="ExternalInput", name="bqc", uniquify=False)
            bkc_d = dram.tile([128, NT], f32, kind="ExternalInput", name="bkc", uniquify=False)
            cs_d = dram.tile([128, SEQ], bf, kind="ExternalInput", name="cs2", uniquify=False)
            sn_d = dram.tile([128, SEQ], bf, kind="ExternalInput", name="sn2", uniquify=False)
            mk_d = dram.tile([128, 256], bf, kind="ExternalInput", name="msk", uniquify=False)
            outT_d = dram.tile([DH, SEQ], f32, kind="ExternalOutput", name="outT", uniquify=False)
            l_d = dram.tile([HPC, SEQ], f32, kind="ExternalOutput", name="lsum", uniquify=False)

            with tc.tile_pool(name="const", bufs=1) as cp, \
                 tc.tile_pool(name="xq_p", bufs=8) as xqp, \
                 tc.tile_pool(name="xk_p", bufs=8) as xkp, \
                 tc.tile_pool(name="xv_p", bufs=8) as xvp, \
                 tc.tile_pool(name="rope", bufs=3) as rp, \
                 tc.tile_pool(name="ostage", bufs=3) as op, \
                 tc.tile_pool(name="pp", bufs=2, space="PSUM") as pp, \
                 tc.tile_pool(name="sp", bufs=2, space="PSUM") as sp, \
                 tc.tile_pool(name="tA", bufs=1, space="PSUM") as ptA, \
                 tc.tile_pool(name="tB", bufs=1, space="PSUM") as ptB:

                qh = cp.tile([128, NT, SEQ], bf, name="qh")
                kh = cp.tile([128, NT, SEQ], bf, name="kh")
                vsb = cp.tile([128, NKT, HPC * 65], bf, name="vsb")
                probs = cp.tile([128, 2, NKT, 512], bf, name="probs")
                w_sb = {n: cp.tile([128, NE, DH], bf, name=f"w_{n}") for n in "qkv"}
                b_sb = {n: cp.tile([128, NT], f32, name=f"b_{n}") for n in "qk"}
                cs = cp.tile([128, SEQ], bf, name="cs")
                sn = cp.tile([128, SEQ], bf, name="sn")
                msk = cp.tile([128, 2, 128], bf, name="mskt")

                # ---------------- DMA issue (queued upfront, 3 queues) ----
                # x inputs in [128, 512] sc-chunks so the first matmul's
                # operands land within ~7us and pool live-set stays small.
                xt = {"q": {}, "k": {}, "v": {}}
                pools = {"q": xqp, "k": xkp, "v": xvp}
                srcs = {"q": xq_d, "k": xk_d, "v": xv_d}

                def load_x(nm, e, h, eng):
                    # [128, 1024] half-seq tiles: 256KB descriptors are ~2x
                    # more byte-efficient on the DMA path than 128KB ones
                    t = pools[nm].tile([128, 1024], bf, tag="x", name=f"x{nm}{e}_{h}")
                    eng.dma_start(out=t[:, :],
                                  in_=srcs[nm][e * 128:(e + 1) * 128,
                                               h * 1024:(h + 1) * 1024])
                    xt[nm][(e, h)] = t

                wd = {"q": wq_d, "k": wk_d, "v": wv_d}

                def load_w(nm, e, eng):
                    eng.dma_start(out=w_sb[nm][:, e, :],
                                  in_=wd[nm][e * 128:(e + 1) * 128, :])

                # DMA plan. Two fast queues run in parallel (gpsimd DMA is
                # ~2.3us/descriptor -- useless). Scalar gets the weights and
                # v inputs (its exp work only starts at the attention
                # phase); sync gets the x streams. All pool-recycled (hence
                # dependency-gated) sc2/3 loads sit at the sync tail in
                # need-order so they can't head-of-line-block anything:
                # their gates only depend on wave-1 PE work. Output stores
                # are appended inline later.
                for e in range(NE):
                    load_w("q", e, nc.scalar)
                    load_x("q", e, 0, nc.sync)
                nc.scalar.dma_start(out=b_sb["q"][:, :], in_=bqc_d[:, :])
                nc.scalar.dma_start(out=b_sb["k"][:, :], in_=bkc_d[:, :])
                nc.sync.dma_start(out=cs[:, :], in_=cs_d[:, :])
                nc.sync.dma_start(out=sn[:, :], in_=sn_d[:, :])
                for e in range(NE):
                    load_w("v", e, nc.sync)
                for e in range(NE):
                    load_x("v", e, 0, nc.sync)
                # k-side loads ride the scalar queue inline with the Q-wave
                # evictions (see the schedule below); h1 halves are
                # pool-recycled and dependency-gated on wave-1 PE consumers
                # only, placed at the sync tail in need-order.
                for e in range(NE):
                    load_x("q", e, 1, nc.sync)
                nc.sync.dma_start(out=msk[:, :, :],
                                  in_=mk_d[:, :].rearrange("p (h u) -> p h u", h=2))
                for e in range(NE):
                    load_x("v", e, 1, nc.sync)

                # only the ones-columns (col 64 of each 65-block) need init;
                # the v evictions overwrite the 64 data columns of every block
                nc.vector.memset(
                    vsb[:, :, :].rearrange("p k (h u) -> p k h u", u=65)[:, :, :, 64:65],
                    1.0)

                # ---------------- projection pieces ----------------------
                # tmp tiles per (nm, m, half): filled by two sc pieces, then
                # roped into qh/kh and released.
                tmps = {}

                def proj_piece(nm, m, sc):
                    # one [128out, 512seq] quantum: 8 e-tile matmul pairs
                    h, c = sc // 2, sc % 2
                    key = (nm, m, h)
                    if key not in tmps:
                        tmps[key] = rp.tile([128, 1024], bf, tag="tmp", bufs=10,
                                            name=f"tp{nm}{m}{h}")
                    tmp = tmps[key]
                    ps = pp.tile([128, 512], f32, tag="p", name=f"pp{nm}{m}{sc}")
                    for e in range(NE):
                        nc.tensor.matmul(
                            ps[0:64, :],
                            w_sb[nm][:, e, m * 128:m * 128 + 64],
                            xt[nm][(e, h)][:, c * 512:(c + 1) * 512],
                            start=(e == 0), stop=(e == NE - 1))
                        nc.tensor.matmul(
                            ps[64:128, :],
                            w_sb[nm][:, e, m * 128 + 64:(m + 1) * 128],
                            xt[nm][(e, h)][:, c * 512:(c + 1) * 512],
                            start=(e == 0), stop=(e == NE - 1))
                    # PSUM eviction + bias: Scalar while it is idle (wave 1),
                    # DVE once exps occupy the scalar queue (wave 2) -- an
                    # eviction queued behind pending exps would head-of-line
                    # stall the pp rotation.
                    if h == 0:
                        nc.scalar.activation(
                            tmp[:, c * 512:(c + 1) * 512], ps[:, :], IDN,
                            bias=b_sb[nm][:, m:m + 1])
                    else:
                        nc.vector.tensor_scalar_add(
                            tmp[:, c * 512:(c + 1) * 512], ps[:, :],
                            b_sb[nm][:, m:m + 1])

                def rope_half(nm, m, h):
                    # consumes tmp(nm, m, h) -> writes dst[:, m, h*1024:+1024]
                    dst = qh if nm == "q" else kh
                    tmp = tmps.pop((nm, m, h))
                    lo, hi = h * 1024, (h + 1) * 1024
                    tsw = rp.tile([128, 1024], bf, tag="tsw", name=f"tw{nm}{m}{h}")
                    for blk in range(4):
                        s = blk ^ 1
                        nc.scalar.dma_start(out=tsw[blk * 32:(blk + 1) * 32, :],
                                            in_=tmp[s * 32:(s + 1) * 32, :])
                    m2 = rp.tile([128, 1024], bf, tag="m2", name=f"m2{nm}{m}{h}")
                    nc.vector.tensor_tensor(dst[:, m, lo:hi], tmp[:, :], cs[:, lo:hi], MUL)
                    nc.vector.tensor_tensor(m2[:, :], tsw[:, :], sn[:, lo:hi], MUL)
                    nc.vector.tensor_tensor(dst[:, m, lo:hi], dst[:, m, lo:hi], m2[:, :], ADD)

                def v_piece(sp2):
                    # two st (128-seq) tiles of the v projection
                    for u in range(2):
                        st = 2 * sp2 + u
                        o = st * 128
                        h, col = o // 1024, o % 1024
                        ps = pp.tile([128, 512], f32, tag="p", name=f"ppv{st}")
                        for e in range(NE):
                            nc.tensor.matmul(
                                ps[0:64, :],
                                xt["v"][(e, h)][:, col:col + 64],
                                w_sb["v"][:, e, :],
                                start=(e == 0), stop=(e == NE - 1))
                            nc.tensor.matmul(
                                ps[64:128, :],
                                xt["v"][(e, h)][:, col + 64:col + 128],
                                w_sb["v"][:, e, :],
                                start=(e == 0), stop=(e == NE - 1))
                        vdst = vsb[:, st, :].rearrange("p (h u) -> p h u", u=65)[:, :, 0:64]
                        vsrc = ps[:, :].rearrange("p (h d) -> p h d", d=64)
                        if h == 0:
                            nc.scalar.activation(vdst, vsrc, CPY)
                        else:
                            nc.vector.tensor_copy(vdst, vsrc)

                # ---------------- filler plumbing ------------------------
                def emit(tok):
                    kind = tok[0]
                    if kind == "Q":
                        proj_piece("q", tok[1], tok[2])
                    elif kind == "K":
                        proj_piece("k", tok[1], tok[2])
                    elif kind == "V":
                        v_piece(tok[1])
                    elif kind == "RQ":
                        rope_half("q", tok[1], tok[2])
                    elif kind == "RK":
                        rope_half("k", tok[1], tok[2])
                    done.add(tok)

                done = set()

                # ---------------- attention unit steps -------------------
                pt = {}

                # wave-2 filler pump: projection pieces decomposed into
                # 2-e-tile chunks (~0.44us) emitted one per kt, small enough
                # to fit the exp-pacing slack without stalling the scores.
                pump_q = deque()

                def piece_gen(nm, m, sc):
                    h, c = sc // 2, sc % 2
                    key = (nm, m, h)
                    if key not in tmps:
                        tmps[key] = rp.tile([128, 1024], bf, tag="tmp", bufs=10,
                                            name=f"tp{nm}{m}{h}")
                    tmp = tmps[key]
                    ps = pp.tile([128, 512], f32, tag="p", name=f"pp{nm}{m}{sc}")
                    for e0 in range(0, NE, 2):
                        for e in (e0, e0 + 1):
                            nc.tensor.matmul(
                                ps[0:64, :],
                                w_sb[nm][:, e, m * 128:m * 128 + 64],
                                xt[nm][(e, h)][:, c * 512:(c + 1) * 512],
                                start=(e == 0), stop=(e == NE - 1))
                            nc.tensor.matmul(
                                ps[64:128, :],
                                w_sb[nm][:, e, m * 128 + 64:(m + 1) * 128],
                                xt[nm][(e, h)][:, c * 512:(c + 1) * 512],
                                start=(e == 0), stop=(e == NE - 1))
                        if e0 + 2 < NE:
                            yield
                    nc.vector.tensor_scalar_add(
                        tmp[:, c * 512:(c + 1) * 512], ps[:, :],
                        b_sb[nm][:, m:m + 1])

                def v_gen(sp2):
                    for u in range(2):
                        st = 2 * sp2 + u
                        o = st * 128
                        h, col = o // 1024, o % 1024
                        ps = pp.tile([128, 512], f32, tag="p", name=f"ppv{st}")
                        for e0 in range(0, NE, 2):
                            for e in (e0, e0 + 1):
                                nc.tensor.matmul(
                                    ps[0:64, :],
                                    xt["v"][(e, h)][:, col:col + 64],
                                    w_sb["v"][:, e, :],
                                    start=(e == 0), stop=(e == NE - 1))
                                nc.tensor.matmul(
                                    ps[64:128, :],
                                    xt["v"][(e, h)][:, col + 64:col + 128],
                                    w_sb["v"][:, e, :],
                                    start=(e == 0), stop=(e == NE - 1))
                            if not (u == 1 and e0 + 2 >= NE):
                                yield
                        nc.vector.tensor_copy(
                            vsb[:, st, :].rearrange("p (h u) -> p h u", u=65)[:, :, 0:64],
                            ps[:, :].rearrange("p (h d) -> p h d", d=64))

                def rope_gen(nm, m, h):
                    rope_half(nm, m, h)
                    return
                    yield  # pragma: no cover

                def pump(n=1):
                    # advance the head generator by one chunk; a generator's
                    # final chunk is emitted during the next() that raises
                    # StopIteration, so pop-and-stop, never double-advance
                    for _ in range(n):
                        if not pump_q:
                            return
                        try:
                            next(pump_q[0])
                        except StopIteration:
                            pump_q.popleft()

                def unit_kt(t, j, kt, nkt):
                    if kt == 0:
                        pt[0] = ptA.tile([65, 512], f32, tag="t0", name=f"pt0_{t}{j}")
                        pt[1] = ptB.tile([65, 512], f32, tag="t1", name=f"pt1_{t}{j}")
                    # causal column shrink: for diagonal tiles, query columns
                    # below o are fully masked -- skip them in the scores
                    # matmul (moving N), the exp, and the attnV accumulation
                    # (nested ranges, so PSUM accumulate stays consistent).
                    dd = kt - 4 * j if causal else -1
                    o = 128 * max(dd, 0)
                    ps = sp.tile([128, 1024], f32, tag="s", name=f"ps{t}{j}_{kt}")
                    # A/B heads write the tile's two different PSUM banks
                    # from PE row groups 0/1 -> they run concurrently.
                    for half in (0, 1):
                        po = half * 64
                        nc.tensor.matmul(
                            ps[:, half * 512 + o:(half + 1) * 512],
                            kh[po:po + 64, t, kt * 128:(kt + 1) * 128],
                            qh[po:po + 64, t, j * 512 + o:(j + 1) * 512],
                            start=True, stop=True)
                    nc.scalar.activation(
                        probs[:, :, kt, o:],
                        ps[:, :].rearrange("p (h u) -> p h u", h=2)[:, :, o:], EXP)
                    if causal and 0 <= dd:
                        # triangle mask on the single partial 128-col block
                        nc.vector.tensor_tensor(
                            probs[:, :, kt, o:o + 128], probs[:, :, kt, o:o + 128],
                            msk[:, :, :], MUL)
                    for half in (0, 1):
                        lh = 2 * t + half
                        nc.tensor.matmul(
                            pt[half][:, o:],
                            vsb[:, kt, lh * 65:(lh + 1) * 65],
                            probs[:, half, kt, o:],
                            start=(kt == 0), stop=(kt == nkt - 1))

                def unit_end(t, j):
                    for half in (0, 1):
                        lh = 2 * t + half
                        ost = op.tile([65, 512], f32, tag="ost", name=f"os{half}_{t}{j}")
                        nc.vector.tensor_copy(ost[:, :], pt[half][:, :])
                        nc.sync.dma_start(
                            out=outT_d[lh * 64:(lh + 1) * 64, j * 512:(j + 1) * 512],
                            in_=ost[0:64, :])
                        nc.sync.dma_start(
                            out=l_d[lh:lh + 1, j * 512:(j + 1) * 512],
                            in_=ost[64:65, :])

                # ---------------- the schedule ---------------------------
                # Phase 1: all projections as one contiguous hot matmul
                # stream (the PE only reaches full clock in long
                # uninterrupted bursts). Rope/evictions ride on DVE behind.
                # sc-wave-major order: all consumers of the sc0/1 x-chunks
                # run before any sc2/3 piece, so the x pools recycle without
                # blocking the in-order PE queue.
                def unit(t, j):
                    nkt = 4 * (j + 1) if causal else NKT
                    for kt in range(nkt):
                        unit_kt(t, j, kt, nkt)
                    unit_end(t, j)

                # Wave 1: q, v, k projections as contiguous hot matmul
                # streams (the PE only reaches full clock in long bursts).
                for m in range(NT):
                    emit(("Q", m, 0)); emit(("Q", m, 1)); emit(("RQ", m, 0))
                    # k loads woven into the scalar queue's idle slots
                    load_w("k", 2 * m, nc.scalar); load_w("k", 2 * m + 1, nc.scalar)
                    load_x("k", 2 * m, 0, nc.scalar); load_x("k", 2 * m + 1, 0, nc.scalar)
                for sp2 in range(4):
                    emit(("V", sp2))
                for m in range(NT):
                    emit(("K", m, 0)); emit(("K", m, 1)); emit(("RK", m, 0))

                # Wave 2, then the pure exp-paced attention stream.
                for m in range(NT):
                    load_x("k", 2 * m, 1, nc.sync)
                    load_x("k", 2 * m + 1, 1, nc.sync)
                    emit(("Q", m, 2)); emit(("Q", m, 3)); emit(("RQ", m, 1))
                for sp2 in range(4, NKT // 2):
                    emit(("V", sp2))
                for m in range(NT):
                    emit(("K", m, 2)); emit(("K", m, 3)); emit(("RK", m, 1))
                for (t, j) in [(t, j) for j in range(NQC) for t in range(NT)]:
                    unit(t, j)
    _built[causal] = nc
    nc.compile()
    return nc


def _prep_core_inputs(c, q, k, v, Wq, bq, Wk, bk, Wv, bv, sin, cos):
    b, hh = c // 2, c % 2
    hs = slice(hh * DH, (hh + 1) * DH)

    perm = np.empty(DH, np.int64)
    for lh in range(HPC):
        base = (hh * HPC + lh) * HD
        perm[lh * HD:lh * HD + HALF] = base + 2 * np.arange(HALF)
        perm[lh * HD + HALF:(lh + 1) * HD] = base + 2 * np.arange(HALF) + 1

    s = 0.125  # 1/sqrt(HD), folded into the q projection
    wq = np.ascontiguousarray((Wq[perm, :] * s).T).astype(BF16)
    wk = np.ascontiguousarray(Wk[perm, :].T).astype(BF16)
    wv = np.ascontiguousarray(Wv[hs, :].T).astype(BF16)

    p32 = np.arange(128) % 32
    cs2 = cos[:, p32].T.astype(BF16)
    sgn = np.where((np.arange(128) // 32) % 2 == 0, -1.0, 1.0).astype(np.float32)
    sn2 = (sin[:, p32] * sgn[None, :]).T.astype(BF16)

    kk = np.arange(128)[:, None]
    qq = np.arange(128)[None, :]
    tri = (kk <= qq)  # [128, 128] triangle for the partial diagonal block
    msk = np.repeat(tri[:, None, :], 2, axis=1).reshape(128, 256).astype(BF16)

    return {
        "xq": np.ascontiguousarray(q[b].T).astype(BF16),
        "xk": np.ascontiguousarray(k[b].T).astype(BF16),
        "xv": np.ascontiguousarray(v[b].T).astype(BF16),
        "wq": wq, "wk": wk, "wv": wv,
        "bqc": np.ascontiguousarray((bq[perm] * s).reshape(NT, 128).T, np.float32),
        "bkc": np.ascontiguousarray(bk[perm].reshape(NT, 128).T, np.float32),
        "cs2": cs2, "sn2": sn2, "msk": msk,
    }


def prep_in_maps(q, k, v, Wq, bq, Wk, bk, Wv, bv, sin, cos):
    args = [np.asarray(a, np.float32) for a in (q, k, v, Wq, bq, Wk, bk, Wv, bv, sin, cos)]
    maps = [_prep_core_inputs(c, *args) for c in range(8)]
    return maps, args[8]  # bv needed on host in assemble()


def assemble(results, bv):
    out = np.empty((BATCH, SEQ, EMB), np.float32)
    for c in range(8):
        b, hh = c // 2, c % 2
        outT = np.asarray(results[c]["outT"], np.float32)
        l = np.asarray(results[c]["lsum"], np.float32)
        a = outT.reshape(HPC, HD, SEQ) / l[:, None, :]
        out[b, :, hh * DH:(hh + 1) * DH] = a.reshape(DH, SEQ).T \
            + bv[hh * DH:(hh + 1) * DH][None, :]
    return out


def run(in_maps, causal=True, trace=False, **kw):
    _install_ntff_shim()
    from concourse.bass_utils import run_bass_kernel_spmd
    nc = build(causal)
    return run_bass_kernel_spmd(nc, in_maps, core_ids=list(range(8)), trace=trace, **kw)


def kernel(q, k, v, Wq, bq, Wk, bk, Wv, bv, sin, cos, mask):
    in_maps, bv_f = prep_in_maps(q, k, v, Wq, bq, Wk, bk, Wv, bv, sin, cos)
    r = run(in_maps, causal=bool(mask))
    return assemble(r.results, bv_f)


# revision 52
# speedup vs baseline: 1.0775x; 1.0775x over previous
"""Multi-head causal attention (QKV proj + RoPE + softmax) on 8 TRN2 NeuronCores.

Sharding: batch 4-way x head-group 2-way -> each core handles 1 batch and 8
contiguous heads (512 output channels). No collectives; host gathers slices.

Per-core algorithm (all matmul compute in bf16, fp32 PSUM accumulation):
  - host passes x.T (q/k/v of its batch, transposed to [emb, seq]) and W.T
    shards so every matmul contracts over the partition dim without on-device
    transposes.
  - q/k weights are row-permuted per head into [even dims | odd dims] so RoPE
    becomes: rot = x*cs + swap32(x)*sn, where swap32 is an SBUF partition-block
    swap done by DMA. The per-head dim permutation cancels in q.k dot products.
  - q/k biases are per-partition columns folded into the PSUM eviction
    (tensor_scalar add); the v bias is applied on host: P@(V+b) = P@V + l*b.
  - scores are computed transposed, S_T[k, q] = kh_T.T @ qh_T (K=64
    contraction; the A/B heads of a 128-row tile are emitted adjacently so
    they run concurrently on PE row groups).
  - softmax: exp on ScalarE from PSUM at [128,1024] granularity (no max
    subtraction: |scores| <= ~5 by construction), causal mask multiplies on
    DVE for the diagonal tiles only; fully-masked k-tiles are skipped.
  - attnT[d, q] = sum_kt V_tile[k,d|1].T @ P_T[k, q] -- a ones-column appended
    to V makes row 64 the softmax denominator for free.
  - unnormalized attnT and the denominator row go to HBM via one SBUF staging
    copy; division + final transpose + v-bias happen on host.

Scheduling (the perf rewrite vs the 313us baseline): instead of projections
-> attention phases, everything is one software pipeline. Attention units
start as soon as the m=0 projections land (~19us), and the remaining
projection pieces (8 matmul-pair quanta) are woven between the per-kt
score/exp/attnV steps so the PE never idles while ScalarE paces the exp
stream. DMA issue is split across the sync/gpsimd/vector queues so no
engine serializes input loading.
"""

import sys
import types
from collections import deque

import numpy as np
import ml_dtypes

BF16 = ml_dtypes.bfloat16
SEQ, EMB, NHEADS, BATCH = 2048, 1024, 16, 4
HD, HALF = 64, 32
HPC = 8          # heads per core
DH = 512         # output dims per core
NE = EMB // 128  # 8 contraction tiles
NT = 4           # head-pair (128-row) dout tiles
NKT = SEQ // 128  # 16 key tiles
NQC = SEQ // 512  # 4 query chunks


def _install_ntff_shim():
    """The image's antenv lacks axon_hooks; synthesize it from trn_agent_boot
    so run_bass_kernel_spmd(trace=True) can profile. Harmless if unused."""
    try:
        import antenv.axon_hooks  # noqa: F401
        return
    except ImportError:
        pass
    try:
        from trn_agent_boot.trn_boot import _ntff_profile_via_ctypes
        import antenv
    except ImportError:
        return
    hook = _ntff_profile_via_ctypes("/opt/axon/libaxon_pjrt.so")
    mod = types.ModuleType("antenv.axon_hooks")
    mod.get_axon_ntff_profile_hook = lambda: hook
    mod.set_axon_ntff_profile_hook = lambda h: None
    sys.modules["antenv.axon_hooks"] = mod
    antenv.axon_hooks = mod


_built = {}


def build(causal=True):
    if causal in _built:
        return _built[causal]
    import concourse.mybir as mybir
    import concourse.tile as tile
    from concourse import bacc

    f32 = mybir.dt.float32
    bf = mybir.dt.bfloat16
    EXP = mybir.ActivationFunctionType.Exp
    IDN = mybir.ActivationFunctionType.Identity
    CPY = mybir.ActivationFunctionType.Copy
    MUL = mybir.AluOpType.mult
    ADD = mybir.AluOpType.add

    nc = bacc.Bacc(None, target_bir_lowering=False, debug=False)
    with tile.TileContext(nc) as tc:
        with tc.tile_pool(name="dram", bufs=1, space="DRAM") as dram:
            xq_d = dram.tile([EMB, SEQ], bf, kind="ExternalInput", name="xq", uniquify=False)
            xk_d = dram.tile([EMB, SEQ], bf, kind="ExternalInput", name="xk", uniquify=False)
            xv_d = dram.tile([EMB, SEQ], bf, kind="ExternalInput", name="xv", uniquify=False)
            wq_d = dram.tile([EMB, DH], bf, kind="ExternalInput", name="wq", uniquify=False)
            wk_d = dram.tile([EMB, DH], bf, kind="ExternalInput", name="wk", uniquify=False)
            wv_d = dram.tile([EMB, DH], bf, kind="ExternalInput", name="wv", uniquify=False)
            bqc_d = dram.tile([128, NT], f32, kind="ExternalInput", name="bqc", uniquify=False)
            bkc_d = dram.tile([128, NT], f32, kind="ExternalInput", name="bkc", uniquify=False)
            cs_d = dram.tile([128, SEQ], bf, kind="ExternalInput", name="cs2", uniquify=False)
            sn_d = dram.tile([128, SEQ], bf, kind="ExternalInput", name="sn2", uniquify=False)
            mk_d = dram.tile([128, 256], bf, kind="ExternalInput", name="msk", uniquify=False)
            outT_d = dram.tile([DH, SEQ], f32, kind="ExternalOutput", name="outT", uniquify=False)
            l_d = dram.tile([HPC, SEQ], f32, kind="ExternalOutput", name="lsum", uniquify=False)

            with tc.tile_pool(name="const", bufs=1) as cp, \
                 tc.tile_pool(name="xq_p", bufs=8) as xqp, \
                 tc.tile_pool(name="xk_p", bufs=8) as xkp, \
                 tc.tile_pool(name="xv_p", bufs=8) as xvp, \
                 tc.tile_pool(name="rope", bufs=3) as rp, \
                 tc.tile_pool(name="ostage", bufs=3) as op, \
                 tc.tile_pool(name="pp", bufs=2, space="PSUM") as pp, \
                 tc.tile_pool(name="sp", bufs=2, space="PSUM") as sp, \
                 tc.tile_pool(name="tA", bufs=1, space="PSUM") as ptA, \
                 tc.tile_pool(name="tB", bufs=1, space="PSUM") as ptB:

                qh = cp.tile([128, NT, SEQ], bf, name="qh")
                kh = cp.tile([128, NT, SEQ], bf, name="kh")
                vsb = cp.tile([128, NKT, HPC * 65], bf, name="vsb")
                probs = cp.tile([128, 2, NKT, 512], bf, name="probs")
                w_sb = {n: cp.tile([128, NE, DH], bf, name=f"w_{n}") for n in "qkv"}
                b_sb = {n: cp.tile([128, NT], f32, name=f"b_{n}") for n in "qk"}
                cs = cp.tile([128, SEQ], bf, name="cs")
                sn = cp.tile([128, SEQ], bf, name="sn")
                msk = cp.tile([128, 2, 128], bf, name="mskt")

                # ---------------- DMA issue (queued upfront, 3 queues) ----
                # x inputs in [128, 512] sc-chunks so the first matmul's
                # operands land within ~7us and pool live-set stays small.
                xt = {"q": {}, "k": {}, "v": {}}
                pools = {"q": xqp, "k": xkp, "v": xvp}
                srcs = {"q": xq_d, "k": xk_d, "v": xv_d}

                def load_x(nm, e, h, eng):
                    # [128, 1024] half-seq tiles: 256KB descriptors are ~2x
                    # more byte-efficient on the DMA path than 128KB ones
                    t = pools[nm].tile([128, 1024], bf, tag="x", name=f"x{nm}{e}_{h}")
                    eng.dma_start(out=t[:, :],
                                  in_=srcs[nm][e * 128:(e + 1) * 128,
                                               h * 1024:(h + 1) * 1024])
                    xt[nm][(e, h)] = t

                wd = {"q": wq_d, "k": wk_d, "v": wv_d}

                def load_w(nm, e, eng):
                    eng.dma_start(out=w_sb[nm][:, e, :],
                                  in_=wd[nm][e * 128:(e + 1) * 128, :])

                # DMA plan. Two fast queues run in parallel (gpsimd DMA is
                # ~2.3us/descriptor -- useless). Scalar gets the weights and
                # v inputs (its exp work only starts at the attention
                # phase); sync gets the x streams. All pool-recycled (hence
                # dependency-gated) sc2/3 loads sit at the sync tail in
                # need-order so they can't head-of-line-block anything:
                # their gates only depend on wave-1 PE work. Output stores
                # are appended inline later.
                for e in range(NE):
                    load_w("q", e, nc.scalar)
                    load_x("q", e, 0, nc.sync)
                nc.scalar.dma_start(out=b_sb["q"][:, :], in_=bqc_d[:, :])
                nc.scalar.dma_start(out=b_sb["k"][:, :], in_=bkc_d[:, :])
                nc.sync.dma_start(out=cs[:, :], in_=cs_d[:, :])
                nc.sync.dma_start(out=sn[:, :], in_=sn_d[:, :])
                for e in range(NE):
                    load_w("v", e, nc.sync)
                for e in range(NE):
                    load_x("v", e, 0, nc.sync)
                # k-side loads ride the scalar queue inline with the Q-wave
                # evictions (see the schedule below); h1 halves are
                # pool-recycled and dependency-gated on wave-1 PE consumers
                # only, placed at the sync tail in need-order.
                for e in range(NE):
                    load_x("q", e, 1, nc.sync)
                nc.sync.dma_start(out=msk[:, :, :],
                                  in_=mk_d[:, :].rearrange("p (h u) -> p h u", h=2))
                for e in range(NE):
                    load_x("v", e, 1, nc.sync)

                # only the ones-columns (col 64 of each 65-block) need init;
                # the v evictions overwrite the 64 data columns of every block
                nc.vector.memset(
                    vsb[:, :, :].rearrange("p k (h u) -> p k h u", u=65)[:, :, :, 64:65],
                    1.0)

                # ---------------- projection pieces ----------------------
                # tmp tiles per (nm, m, half): filled by two sc pieces, then
                # roped into qh/kh and released.
                tmps = {}

                def proj_piece(nm, m, sc):
                    # one [128out, 512seq] quantum: 8 e-tile matmul pairs
                    h, c = sc // 2, sc % 2
                    key = (nm, m, h)
                    if key not in tmps:
                        tmps[key] = rp.tile([128, 1024], bf, tag="tmp", bufs=10,
                                            name=f"tp{nm}{m}{h}")
                    tmp = tmps[key]
                    ps = pp.tile([128, 512], f32, tag="p", name=f"pp{nm}{m}{sc}")
                    for e in range(NE):
                        nc.tensor.matmul(
                            ps[0:64, :],
                            w_sb[nm][:, e, m * 128:m * 128 + 64],
                            xt[nm][(e, h)][:, c * 512:(c + 1) * 512],
                            start=(e == 0), stop=(e == NE - 1))
                        nc.tensor.matmul(
                            ps[64:128, :],
                            w_sb[nm][:, e, m * 128 + 64:(m + 1) * 128],
                            xt[nm][(e, h)][:, c * 512:(c + 1) * 512],
                            start=(e == 0), stop=(e == NE - 1))
                    # PSUM eviction + bias on the Scalar engine (idle during
                    # the projection phase; DVE keeps only the rope muls)
                    nc.scalar.activation(
                        tmp[:, c * 512:(c + 1) * 512], ps[:, :], IDN,
                        bias=b_sb[nm][:, m:m + 1])

                def rope_half(nm, m, h):
                    # consumes tmp(nm, m, h) -> writes dst[:, m, h*1024:+1024]
                    dst = qh if nm == "q" else kh
                    tmp = tmps.pop((nm, m, h))
                    lo, hi = h * 1024, (h + 1) * 1024
                    tsw = rp.tile([128, 1024], bf, tag="tsw", name=f"tw{nm}{m}{h}")
                    for blk in range(4):
                        s = blk ^ 1
                        nc.scalar.dma_start(out=tsw[blk * 32:(blk + 1) * 32, :],
                                            in_=tmp[s * 32:(s + 1) * 32, :])
                    m2 = rp.tile([128, 1024], bf, tag="m2", name=f"m2{nm}{m}{h}")
                    nc.vector.tensor_tensor(dst[:, m, lo:hi], tmp[:, :], cs[:, lo:hi], MUL)
                    nc.vector.tensor_tensor(m2[:, :], tsw[:, :], sn[:, lo:hi], MUL)
                    nc.vector.tensor_tensor(dst[:, m, lo:hi], dst[:, m, lo:hi], m2[:, :], ADD)

                def v_piece(sp2):
                    # two st (128-seq) tiles of the v projection
                    for u in range(2):
                        st = 2 * sp2 + u
                        o = st * 128
                        h, col = o // 1024, o % 1024
                        ps = pp.tile([128, 512], f32, tag="p", name=f"ppv{st}")
                        for e in range(NE):
                            nc.tensor.matmul(
                                ps[0:64, :],
                                xt["v"][(e, h)][:, col:col + 64],
                                w_sb["v"][:, e, :],
                                start=(e == 0), stop=(e == NE - 1))
                            nc.tensor.matmul(
                                ps[64:128, :],
                                xt["v"][(e, h)][:, col + 64:col + 128],
                                w_sb["v"][:, e, :],
                                start=(e == 0), stop=(e == NE - 1))
                        nc.scalar.activation(
                            vsb[:, st, :]
                            .rearrange("p (h u) -> p h u", u=65)[:, :, 0:64],
                            ps[:, :].rearrange("p (h d) -> p h d", d=64), CPY)

                # ---------------- filler plumbing ------------------------
                def emit(tok):
                    kind = tok[0]
                    if kind == "Q":
                        proj_piece("q", tok[1], tok[2])
                    elif kind == "K":
                        proj_piece("k", tok[1], tok[2])
                    elif kind == "V":
                        v_piece(tok[1])
                    elif kind == "RQ":
                        rope_half("q", tok[1], tok[2])
                    elif kind == "RK":
                        rope_half("k", tok[1], tok[2])
                    done.add(tok)

                done = set()

                # ---------------- attention unit steps -------------------
                pt = {}

                # wave-2 filler pump: projection pieces decomposed into
                # 2-e-tile chunks (~0.44us) emitted one per kt, small enough
                # to fit the exp-pacing slack without stalling the scores.
                pump_q = deque()

                def piece_gen(nm, m, sc):
                    h, c = sc // 2, sc % 2
                    key = (nm, m, h)
                    if key not in tmps:
                        tmps[key] = rp.tile([128, 1024], bf, tag="tmp", bufs=10,
                                            name=f"tp{nm}{m}{h}")
                    tmp = tmps[key]
                    ps = pp.tile([128, 512], f32, tag="p", name=f"pp{nm}{m}{sc}")
                    for e0 in range(0, NE, 2):
                        for e in (e0, e0 + 1):
                            nc.tensor.matmul(
                                ps[0:64, :],
                                w_sb[nm][:, e, m * 128:m * 128 + 64],
                                xt[nm][(e, h)][:, c * 512:(c + 1) * 512],
                                start=(e == 0), stop=(e == NE - 1))
                            nc.tensor.matmul(
                                ps[64:128, :],
                                w_sb[nm][:, e, m * 128 + 64:(m + 1) * 128],
                                xt[nm][(e, h)][:, c * 512:(c + 1) * 512],
                                start=(e == 0), stop=(e == NE - 1))
                        if e0 + 2 < NE:
                            yield
                    nc.vector.tensor_scalar_add(
                        tmp[:, c * 512:(c + 1) * 512], ps[:, :],
                        b_sb[nm][:, m:m + 1])

                def v_gen(sp2):
                    for u in range(2):
                        st = 2 * sp2 + u
                        o = st * 128
                        h, col = o // 1024, o % 1024
                        ps = pp.tile([128, 512], f32, tag="p", name=f"ppv{st}")
                        for e0 in range(0, NE, 2):
                            for e in (e0, e0 + 1):
                                nc.tensor.matmul(
                                    ps[0:64, :],
                                    xt["v"][(e, h)][:, col:col + 64],
                                    w_sb["v"][:, e, :],
                                    start=(e == 0), stop=(e == NE - 1))
                                nc.tensor.matmul(
                                    ps[64:128, :],
                                    xt["v"][(e, h)][:, col + 64:col + 128],
                                    w_sb["v"][:, e, :],
                                    start=(e == 0), stop=(e == NE - 1))
                            if not (u == 1 and e0 + 2 >= NE):
                                yield
                        nc.vector.tensor_copy(
                            vsb[:, st, :].rearrange("p (h u) -> p h u", u=65)[:, :, 0:64],
                            ps[:, :].rearrange("p (h d) -> p h d", d=64))

                def rope_gen(nm, m, h):
                    rope_half(nm, m, h)
                    return
                    yield  # pragma: no cover

                def pump(n=1):
                    # advance the head generator by one chunk; a generator's
                    # final chunk is emitted during the next() that raises
                    # StopIteration, so pop-and-stop, never double-advance
                    for _ in range(n):
                        if not pump_q:
                            return
                        try:
                            next(pump_q[0])
                        except StopIteration:
                            pump_q.popleft()

                def unit_kt(t, j, kt, nkt):
                    if kt == 0:
                        pt[0] = ptA.tile([65, 512], f32, tag="t0", name=f"pt0_{t}{j}")
                        pt[1] = ptB.tile([65, 512], f32, tag="t1", name=f"pt1_{t}{j}")
                    # causal column shrink: for diagonal tiles, query columns
                    # below o are fully masked -- skip them in the scores
                    # matmul (moving N), the exp, and the attnV accumulation
                    # (nested ranges, so PSUM accumulate stays consistent).
                    dd = kt - 4 * j if causal else -1
                    o = 128 * max(dd, 0)
                    ps = sp.tile([128, 1024], f32, tag="s", name=f"ps{t}{j}_{kt}")
                    # A/B heads write the tile's two different PSUM banks
                    # from PE row groups 0/1 -> they run concurrently.
                    for half in (0, 1):
                        po = half * 64
                        nc.tensor.matmul(
                            ps[:, half * 512 + o:(half + 1) * 512],
                            kh[po:po + 64, t, kt * 128:(kt + 1) * 128],
                            qh[po:po + 64, t, j * 512 + o:(j + 1) * 512],
                            start=True, stop=True)
                    nc.scalar.activation(
                        probs[:, :, kt, o:],
                        ps[:, :].rearrange("p (h u) -> p h u", h=2)[:, :, o:], EXP)
                    if causal and 0 <= dd:
                        # triangle mask on the single partial 128-col block
                        nc.vector.tensor_tensor(
                            probs[:, :, kt, o:o + 128], probs[:, :, kt, o:o + 128],
                            msk[:, :, :], MUL)
                    for half in (0, 1):
                        lh = 2 * t + half
                        nc.tensor.matmul(
                            pt[half][:, o:],
                            vsb[:, kt, lh * 65:(lh + 1) * 65],
                            probs[:, half, kt, o:],
                            start=(kt == 0), stop=(kt == nkt - 1))

                def unit_end(t, j):
                    for half in (0, 1):
                        lh = 2 * t + half
                        ost = op.tile([65, 512], f32, tag="ost", name=f"os{half}_{t}{j}")
                        nc.vector.tensor_copy(ost[:, :], pt[half][:, :])
                        nc.sync.dma_start(
                            out=outT_d[lh * 64:(lh + 1) * 64, j * 512:(j + 1) * 512],
                            in_=ost[0:64, :])
                        nc.sync.dma_start(
                            out=l_d[lh:lh + 1, j * 512:(j + 1) * 512],
                            in_=ost[64:65, :])

                # ---------------- the schedule ---------------------------
                # Phase 1: all projections as one contiguous hot matmul
                # stream (the PE only reaches full clock in long
                # uninterrupted bursts). Rope/evictions ride on DVE behind.
                # sc-wave-major order: all consumers of the sc0/1 x-chunks
                # run before any sc2/3 piece, so the x pools recycle without
                # blocking the in-order PE queue.
                def unit(t, j):
                    nkt = 4 * (j + 1) if causal else NKT
                    for kt in range(nkt):
                        unit_kt(t, j, kt, nkt)
                    unit_end(t, j)

                # Wave 1: q, v, k projections as contiguous hot matmul
                # streams (the PE only reaches full clock in long bursts).
                for m in range(NT):
                    emit(("Q", m, 0)); emit(("Q", m, 1)); emit(("RQ", m, 0))
                    # k loads woven into the scalar queue's idle slots
                    load_w("k", 2 * m, nc.scalar); load_w("k", 2 * m + 1, nc.scalar)
                    load_x("k", 2 * m, 0, nc.scalar); load_x("k", 2 * m + 1, 0, nc.scalar)
                for sp2 in range(4):
                    emit(("V", sp2))
                for m in range(NT):
                    emit(("K", m, 0)); emit(("K", m, 1)); emit(("RK", m, 0))

                # Wave 2, then the pure exp-paced attention stream.
                for m in range(NT):
                    load_x("k", 2 * m, 1, nc.sync)
                    load_x("k", 2 * m + 1, 1, nc.sync)
                    emit(("Q", m, 2)); emit(("Q", m, 3)); emit(("RQ", m, 1))
                for sp2 in range(4, NKT // 2):
                    emit(("V", sp2))
                for m in range(NT):
                    emit(("K", m, 2)); emit(("K", m, 3)); emit(("RK", m, 1))
                for (t, j) in [(t, j) for j in range(NQC) for t in range(NT)]:
                    unit(t, j)
    _built[causal] = nc
    nc.compile()
    return nc


def _prep_core_inputs(c, q, k, v, Wq, bq, Wk, bk, Wv, bv, sin, cos):
    b, hh = c // 2, c % 2
    hs = slice(hh * DH, (hh + 1) * DH)

    perm = np.empty(DH, np.int64)
    for lh in range(HPC):
        base = (hh * HPC + lh) * HD
        perm[lh * HD:lh * HD + HALF] = base + 2 * np.arange(HALF)
        perm[lh * HD + HALF:(lh + 1) * HD] = base + 2 * np.arange(HALF) + 1

    s = 0.125  # 1/sqrt(HD), folded into the q projection
    wq = np.ascontiguousarray((Wq[perm, :] * s).T).astype(BF16)
    wk = np.ascontiguousarray(Wk[perm, :].T).astype(BF16)
    wv = np.ascontiguousarray(Wv[hs, :].T).astype(BF16)

    p32 = np.arange(128) % 32
    cs2 = cos[:, p32].T.astype(BF16)
    sgn = np.where((np.arange(128) // 32) % 2 == 0, -1.0, 1.0).astype(np.float32)
    sn2 = (sin[:, p32] * sgn[None, :]).T.astype(BF16)

    kk = np.arange(128)[:, None]
    qq = np.arange(128)[None, :]
    tri = (kk <= qq)  # [128, 128] triangle for the partial diagonal block
    msk = np.repeat(tri[:, None, :], 2, axis=1).reshape(128, 256).astype(BF16)

    return {
        "xq": np.ascontiguousarray(q[b].T).astype(BF16),
        "xk": np.ascontiguousarray(k[b].T).astype(BF16),
        "xv": np.ascontiguousarray(v[b].T).astype(BF16),
        "wq": wq, "wk": wk, "wv": wv,
        "bqc": np.ascontiguousarray((bq[perm] * s).reshape(NT, 128).T, np.float32),
        "bkc": np.ascontiguousarray(bk[perm].reshape(NT, 128).T, np.float32),
        "cs2": cs2, "sn2": sn2, "msk": msk,
    }


def prep_in_maps(q, k, v, Wq, bq, Wk, bk, Wv, bv, sin, cos):
    args = [np.asarray(a, np.float32) for a in (q, k, v, Wq, bq, Wk, bk, Wv, bv, sin, cos)]
    maps = [_prep_core_inputs(c, *args) for c in range(8)]
    return maps, args[8]  # bv needed on host in assemble()


def assemble(results, bv):
    out = np.empty((BATCH, SEQ, EMB), np.float32)
    for c in range(8):
        b, hh = c // 2, c % 2
        outT = np.asarray(results[c]["outT"], np.float32)
        l = np.asarray(results[c]["lsum"], np.float32)
        a = outT.reshape(HPC, HD, SEQ) / l[:, None, :]
        out[b, :, hh * DH:(hh + 1) * DH] = a.reshape(DH, SEQ).T \
            + bv[hh * DH:(hh + 1) * DH][None, :]
    return out


def run(in_maps, causal=True, trace=False, **kw):
    _install_ntff_shim()
    from concourse.bass_utils import run_bass_kernel_spmd
    nc = build(causal)
    return run_bass_kernel_spmd(nc, in_maps, core_ids=list(range(8)), trace=trace, **kw)


def kernel(q, k, v, Wq, bq, Wk, bk, Wv, bv, sin, cos, mask):
    in_maps, bv_f = prep_in_maps(q, k, v, Wq, bq, Wk, bk, Wv, bv, sin, cos)
    r = run(in_maps, causal=bool(mask))
    return assemble(r.results, bv_f)
